# revision 4
# baseline (speedup 1.0000x reference)
"""Squared-Euclidean-distance kernel for Trainium2 (8 NeuronCores, SPMD).

Computes out[b,n,u] = sum_d (x[b,n,d] - w[d,u])^2 for
x [8, 4096, 128] f32, w [128, 1024] f32 -> out [8, 4096, 1024] f32,
via the algebraic identity |x|^2 + |w|^2 - 2 x.w.

Distribution: data-parallel over the batch dim — core c handles x[c]
([4096, 128] rows), w replicated. No cross-core communication.

Per-core device kernel (DMA-bound; ~360 GB/s/core across 16 queues):
  - GEMM in fp16 (full PE rate): PSUM = xt_tile.T @ (-2w).
  - Output written to HBM as fp16 (halves the dominant output traffic;
    elementwise error ~1e-3 of scale) and widened to f32 on the host.
    HBM layout is partition-major [128, 32, 1024] so tiles pair into
    one 512 KiB DMA with 4 KiB contiguous descriptors; the host
    de-transposes when unsharding.
  - n-tiles are processed in groups of 2 sharing one PSUM allocation,
    one output buffer, one output DMA, and grouped fp16 fixups.
  - Epilogue split so no engine exceeds the per-group DMA budget
    (~1.46 us): per tile, ScalarE activation(bias=|x|^2) converts cols
    [0:ACT_U) and VectorE does cols [ACT_U:1024) in one
    scalar_tensor_tensor ((acc + x2) + w2); then grouped fp16 +w2
    fixups on ScalarE's slice: GpSimd cols [0:POOL_F), VectorE the
    rest (GpSimd cannot read PSUM, so it only gets SBUF fp16 work).
  - DMAs alternate between the SP and Act hardware DGE queues.
"""

import sys
import types

try:
    import concourse.bass as bass  # noqa: F401
except ImportError:  # fresh interpreter without the repo on sys.path
    sys.path.insert(0, "/opt/trn_rl_repo")

import numpy as np

import concourse.bass as bass
import concourse.bacc as bacc
import concourse.tile as tile
import concourse.mybir as mybir
import concourse.bass_utils as bass_utils
from concourse.bass_utils import run_bass_kernel_spmd

B, N, D, U = 8, 4096, 128, 1024
N_CORES = 8
P = 128
N_TILES = N // P          # 32 n-tiles per core
N_GROUPS = N_TILES // 2   # tiles processed in pairs
U_HALF = 512              # PSUM bank = 512 f32
XT_CHUNK = 512            # xt loaded as 8 chunks of [128, 512]
ACT_U = 576               # epilogue cols on ScalarE (+ fp16 fixups)
POOL_F = 352              # fixup cols on GpSimd
DVE_F_LO, DVE_F_HI = POOL_F, ACT_U   # fixup cols on VectorE

GEMM_DT = mybir.dt.float16
GEMM_NP = np.float16
OUT_DT = mybir.dt.float16
OUT_NP = np.float16


def _install_ntff_hook():
    """Wire the NTFF profile hook the agent image leaves unconnected."""
    if "antenv.axon_hooks" in sys.modules:
        return
    try:
        from trn_agent_boot.trn_boot import _ntff_profile_via_ctypes
        hook = _ntff_profile_via_ctypes("/opt/axon/libaxon_pjrt.so")
    except Exception:
        hook = None
    mod = types.ModuleType("antenv.axon_hooks")
    mod.get_axon_ntff_profile_hook = lambda: hook
    mod.set_axon_ntff_profile_hook = lambda h: None
    sys.modules["antenv.axon_hooks"] = mod
    bass_utils.upload_artifacts = lambda tmpdir: f"local://{tmpdir}"


def build_bass():
    """Build + compile the per-core Bass program (SPMD, same on all cores)."""
    nc = bacc.Bacc("TRN2", target_bir_lowering=False, debug=False,
                   enable_asserts=True, num_devices=N_CORES)

    xt_ap = nc.dram_tensor("xt", [P, N], GEMM_DT, kind="ExternalInput").ap()
    wneg2_ap = nc.dram_tensor("wneg2", [P, U], GEMM_DT, kind="ExternalInput").ap()
    x2_ap = nc.dram_tensor("x2", [P, N_TILES], mybir.dt.float32,
                           kind="ExternalInput").ap()
    # |w_u|^2 broadcast to all partitions, doubled along a tile-pair dim
    w2p2_ap = nc.dram_tensor("w2p2", [P, 2, U], OUT_DT,
                             kind="ExternalInput").ap()
    # partition-major output: out3[p, j, u] = dist[j*128 + p, u]
    out_ap = nc.dram_tensor("out", [P, N_TILES, U], OUT_DT,
                            kind="ExternalOutput").ap()

    add = mybir.AluOpType.add

    with tile.TileContext(nc) as tc:
        with (
            tc.tile_pool(name="singles", bufs=1) as singles,
            tc.tile_pool(name="xchunks", bufs=N // XT_CHUNK) as xchunks,
            tc.tile_pool(name="psum", bufs=2, space="PSUM") as psum_pool,
            tc.tile_pool(name="outs", bufs=4) as out_pool,
        ):
            # Group 0 needs both wneg2 halves + xt chunk 0 + x2 + w2p2;
            # the rest of xt overlaps with compute.
            wneg2_h = []
            for h in range(U // U_HALF):
                wtile = singles.tile([P, U_HALF], GEMM_DT, tag=f"wneg2{h}",
                                     name=f"wneg2{h}")
                wneg2_h.append(wtile)
            nc.sync.dma_start(wneg2_h[0][:], wneg2_ap[:, 0:U_HALF])
            nc.scalar.dma_start(wneg2_h[1][:], wneg2_ap[:, U_HALF:U])
            xt_sbs = []
            for ci in range(N // XT_CHUNK):
                t = xchunks.tile([P, XT_CHUNK], GEMM_DT, tag=f"xt{ci}")
                xt_sbs.append(t)
            nc.sync.dma_start(xt_sbs[0][:], xt_ap[:, 0:XT_CHUNK])
            w2p2_sb = singles.tile([P, 2, U], OUT_DT, tag="w2p2")
            nc.scalar.dma_start(w2p2_sb[:], w2p2_ap[:])
            x2_sb = singles.tile([P, N_TILES], mybir.dt.float32, tag="x2")
            nc.scalar.dma_start(x2_sb[:], x2_ap[:])
            for ci in range(1, N // XT_CHUNK):
                eng = nc.sync if ci % 2 == 0 else nc.scalar
                eng.dma_start(xt_sbs[ci][:],
                              xt_ap[:, ci * XT_CHUNK:(ci + 1) * XT_CHUNK])

            tiles_per_chunk = XT_CHUNK // P
            for g in range(N_GROUPS):
                acc2 = psum_pool.tile([P, 2, U], mybir.dt.float32, tag="acc")
                o2 = out_pool.tile([P, 2, U], OUT_DT, tag="o")
                for t in range(2):
                    j = 2 * g + t
                    chunk = xt_sbs[j // tiles_per_chunk]
                    col0 = (j % tiles_per_chunk) * P
                    lhsT = chunk[:, col0:col0 + P]
                    for h in range(U // U_HALF):
                        nc.tensor.matmul(
                            acc2[:, t, h * U_HALF:(h + 1) * U_HALF],
                            lhsT,
                            wneg2_h[h][:],
                            start=True, stop=True,
                        )
                for t in range(2):
                    j = 2 * g + t
                    x2col = x2_sb[:, j:j + 1]
                    # ScalarE: cols [0:ACT_U) = acc + x2 (bias), f32->fp16.
                    nc.scalar.activation(
                        out=o2[:, t, 0:ACT_U], in_=acc2[:, t, 0:ACT_U],
                        func=mybir.ActivationFunctionType.Identity,
                        bias=x2col, scale=1.0,
                    )
                    # VectorE: cols [ACT_U:U) = (acc + x2) + w2 in one op.
                    nc.vector.scalar_tensor_tensor(
                        out=o2[:, t, ACT_U:U], in0=acc2[:, t, ACT_U:U],
                        scalar=x2col, in1=w2p2_sb[:, t, ACT_U:U],
                        op0=add, op1=add,
                    )
                # Grouped fp16 fixups (+w2) on the ScalarE slice, both tiles.
                nc.gpsimd.tensor_tensor(
                    out=o2[:, :, 0:POOL_F], in0=o2[:, :, 0:POOL_F],
                    in1=w2p2_sb[:, :, 0:POOL_F], op=add,
                )
                nc.vector.tensor_tensor(
                    out=o2[:, :, DVE_F_LO:DVE_F_HI],
                    in0=o2[:, :, DVE_F_LO:DVE_F_HI],
                    in1=w2p2_sb[:, :, DVE_F_LO:DVE_F_HI], op=add,
                )
                eng = nc.sync if g % 2 == 0 else nc.scalar
                eng.dma_start(out_ap[:, 2 * g:2 * g + 2, :], o2[:])

    nc.compile()
    return nc


_CACHED_NC = None


def _get_nc():
    global _CACHED_NC
    if _CACHED_NC is None:
        _CACHED_NC = build_bass()
    return _CACHED_NC


def make_in_maps(x, w):
    """Host-side shard + precompute: per-core input dict list."""
    x = np.asarray(x, dtype=np.float32)
    w = np.asarray(w, dtype=np.float32)
    wneg2 = (-2.0 * w).astype(GEMM_NP)
    w2 = (w.astype(np.float64) ** 2).sum(axis=0).astype(np.float32)
    w2p2 = np.broadcast_to(w2.astype(OUT_NP), (P, 2, U)).copy()  # [128,2,1024]
    in_maps = []
    for c in range(N_CORES):
        xs = x[c]                                    # [4096, 128]
        xt = np.ascontiguousarray(xs.T).astype(GEMM_NP)       # [128, 4096]
        x2 = (xs ** 2).sum(axis=1, dtype=np.float32)          # [4096]
        x2cols = np.ascontiguousarray(x2.reshape(N_TILES, P).T)  # [128, 32]
        in_maps.append({"xt": xt, "wneg2": wneg2, "x2": x2cols, "w2p2": w2p2})
    return in_maps


def run(x, w, trace=False):
    _install_ntff_hook()
    nc = _get_nc()
    in_maps = make_in_maps(x, w)
    last_err = None
    for _attempt in range(3):
        try:
            res = run_bass_kernel_spmd(nc, in_maps,
                                       core_ids=list(range(N_CORES)),
                                       trace=trace)
            break
        except Exception as e:  # transient device/tunnel hiccups
            last_err = e
    else:
        raise last_err
    # out3[p, j, u] -> dist[j*128 + p, u], widened to f32
    outs = []
    for c in range(N_CORES):
        o3 = res.results[c]["out"]                   # [128, 32, 1024] fp16
        outs.append(o3.transpose(1, 0, 2).reshape(N, U).astype(np.float32))
    return np.stack(outs, axis=0), res


def kernel(x, w):
    out, _ = run(x, w, trace=False)
    return out


# revision 11
# speedup vs baseline: 1.4274x; 1.4274x over previous
"""Squared-Euclidean-distance kernel for Trainium2 (8 NeuronCores, SPMD).

Computes out[b,n,u] = sum_d (x[b,n,d] - w[d,u])^2 for
x [8, 4096, 128] f32, w [128, 1024] f32 -> out [8, 4096, 1024] f32,
via the algebraic identity |x|^2 + |w|^2 - 2 x.w.

Distribution: data-parallel over the batch dim — core c handles x[c]
([4096, 128] rows), w replicated. No cross-core communication.

Per-core device kernel (DMA-bound; ~360 GB/s/core across 16 queues):
  - GEMM in fp16 (full PE rate): PSUM = xt_tile.T @ (-2w).
  - Output written to HBM as fp16 (halves the dominant output traffic;
    elementwise error ~1e-3 of scale) and widened to f32 on the host.
  - Epilogue split so no engine exceeds the per-tile DMA budget
    (~730 ns): ScalarE activation(bias=|x|^2) converts cols [0:ACT_U)
    and VectorE does cols [ACT_U:1024) in one scalar_tensor_tensor
    ((acc + x2) + w2); then fp16 +w2 fixups on ScalarE's slice:
    GpSimd cols [0:POOL_F), VectorE the rest (GpSimd cannot read
    PSUM, so it only gets SBUF fp16 work).
  - DMAs alternate between the SP and Act hardware DGE queues.
"""

import sys
import types

try:
    import concourse.bass as bass  # noqa: F401
except ImportError:  # fresh interpreter without the repo on sys.path
    sys.path.insert(0, "/opt/trn_rl_repo")

import numpy as np

import concourse.bass as bass
import concourse.bacc as bacc
import concourse.tile as tile
import concourse.mybir as mybir
import concourse.bass_utils as bass_utils
from concourse.bass_utils import run_bass_kernel_spmd

B, N, D, U = 8, 4096, 128, 1024
N_CORES = 8
P = 128
N_TILES = N // P          # 32 n-tiles per core
U_HALF = 512              # PSUM bank = 512 f32
XT_CHUNK = 512            # xt loaded as 8 chunks of [128, 512]
ACT_U = 576               # epilogue cols on ScalarE (+ fp16 fixups)
POOL_F = 288              # fixup cols on GpSimd: [0:POOL_F)
DVE_F_LO, DVE_F_HI = POOL_F, ACT_U   # fixup cols on VectorE

GEMM_DT = mybir.dt.float16
GEMM_NP = np.float16
OUT_DT = mybir.dt.float16
OUT_NP = np.float16


def _install_ntff_hook():
    """Wire the NTFF profile hook the agent image leaves unconnected."""
    if "antenv.axon_hooks" in sys.modules:
        return
    try:
        from trn_agent_boot.trn_boot import _ntff_profile_via_ctypes
        hook = _ntff_profile_via_ctypes("/opt/axon/libaxon_pjrt.so")
    except Exception:
        hook = None
    mod = types.ModuleType("antenv.axon_hooks")
    mod.get_axon_ntff_profile_hook = lambda: hook
    mod.set_axon_ntff_profile_hook = lambda h: None
    sys.modules["antenv.axon_hooks"] = mod
    bass_utils.upload_artifacts = lambda tmpdir: f"local://{tmpdir}"


def build_bass():
    """Build + compile the per-core Bass program (SPMD, same on all cores)."""
    nc = bacc.Bacc("TRN2", target_bir_lowering=False, debug=False,
                   enable_asserts=True, num_devices=N_CORES)

    xt_ap = nc.dram_tensor("xt", [P, N], GEMM_DT, kind="ExternalInput").ap()
    wneg2_ap = nc.dram_tensor("wneg2", [P, U], GEMM_DT, kind="ExternalInput").ap()
    x2_ap = nc.dram_tensor("x2", [P, N_TILES], mybir.dt.float32,
                           kind="ExternalInput").ap()
    # |w_u|^2 broadcast to all 128 partitions, precomputed on host in fp16
    w2p_ap = nc.dram_tensor("w2p", [P, U], OUT_DT, kind="ExternalInput").ap()
    out_ap = nc.dram_tensor("out", [N, U], OUT_DT,
                            kind="ExternalOutput").ap()

    add = mybir.AluOpType.add

    with tile.TileContext(nc) as tc:
        with (
            tc.tile_pool(name="singles", bufs=1) as singles,
            tc.tile_pool(name="xchunks", bufs=N // XT_CHUNK) as xchunks,
            tc.tile_pool(name="psum", bufs=4, space="PSUM") as psum_pool,
            tc.tile_pool(name="outs", bufs=8) as out_pool,
        ):
            # Group 0 needs both wneg2 halves + xt chunk 0 + x2 + w2p2;
            # the rest of xt overlaps with compute.
            wneg2_h = []
            for h in range(U // U_HALF):
                wtile = singles.tile([P, U_HALF], GEMM_DT, tag=f"wneg2{h}",
                                     name=f"wneg2{h}")
                wneg2_h.append(wtile)
            nc.sync.dma_start(wneg2_h[0][:], wneg2_ap[:, 0:U_HALF])
            nc.scalar.dma_start(wneg2_h[1][:], wneg2_ap[:, U_HALF:U])
            xt_sbs = []
            for ci in range(N // XT_CHUNK):
                t = xchunks.tile([P, XT_CHUNK], GEMM_DT, tag=f"xt{ci}")
                xt_sbs.append(t)
            nc.sync.dma_start(xt_sbs[0][:], xt_ap[:, 0:XT_CHUNK])
            w2p_sb = singles.tile([P, U], OUT_DT, tag="w2p")
            nc.scalar.dma_start(w2p_sb[:], w2p_ap[:])
            x2_sb = singles.tile([P, N_TILES], mybir.dt.float32, tag="x2")
            nc.scalar.dma_start(x2_sb[:], x2_ap[:])
            for ci in range(1, N // XT_CHUNK):
                eng = nc.sync if ci % 2 == 0 else nc.scalar
                eng.dma_start(xt_sbs[ci][:],
                              xt_ap[:, ci * XT_CHUNK:(ci + 1) * XT_CHUNK])

            tiles_per_chunk = XT_CHUNK // P
            for j in range(N_TILES):
                chunk = xt_sbs[j // tiles_per_chunk]
                col0 = (j % tiles_per_chunk) * P
                lhsT = chunk[:, col0:col0 + P]

                acc = psum_pool.tile([P, U], mybir.dt.float32, tag="acc")
                for h in range(U // U_HALF):
                    nc.tensor.matmul(
                        acc[:, h * U_HALF:(h + 1) * U_HALF],
                        lhsT,
                        wneg2_h[h][:],
                        start=True, stop=True,
                    )

                o = out_pool.tile([P, U], OUT_DT, tag="o")
                x2col = x2_sb[:, j:j + 1]
                # ScalarE: cols [0:ACT_U) = acc + x2 (bias), f32->fp16.
                nc.scalar.activation(
                    out=o[:, 0:ACT_U], in_=acc[:, 0:ACT_U],
                    func=mybir.ActivationFunctionType.Identity,
                    bias=x2col, scale=1.0,
                )
                # VectorE: cols [ACT_U:U) = (acc + x2) + w2 in one op.
                nc.vector.scalar_tensor_tensor(
                    out=o[:, ACT_U:U], in0=acc[:, ACT_U:U],
                    scalar=x2col, in1=w2p_sb[:, ACT_U:U],
                    op0=add, op1=add,
                )
                # fp16 fixups (+w2) on the ScalarE slice: GpSimd + VectorE.
                nc.gpsimd.tensor_tensor(
                    out=o[:, 0:POOL_F], in0=o[:, 0:POOL_F],
                    in1=w2p_sb[:, 0:POOL_F], op=add,
                )
                nc.vector.tensor_tensor(
                    out=o[:, DVE_F_LO:DVE_F_HI], in0=o[:, DVE_F_LO:DVE_F_HI],
                    in1=w2p_sb[:, DVE_F_LO:DVE_F_HI], op=add,
                )
                eng = nc.sync if j % 2 == 0 else nc.scalar
                eng.dma_start(out_ap[j * P:(j + 1) * P, :], o[:])

    nc.compile()
    return nc


_CACHED_NC = None


def _get_nc():
    global _CACHED_NC
    if _CACHED_NC is None:
        _CACHED_NC = build_bass()
    return _CACHED_NC


def make_in_maps(x, w):
    """Host-side shard + precompute: per-core input dict list."""
    x = np.asarray(x, dtype=np.float32)
    w = np.asarray(w, dtype=np.float32)
    wneg2 = (-2.0 * w).astype(GEMM_NP)
    w2 = (w.astype(np.float64) ** 2).sum(axis=0).astype(np.float32)
    w2p = np.broadcast_to(w2.astype(OUT_NP), (P, U)).copy()     # [128, 1024]
    in_maps = []
    for c in range(N_CORES):
        xs = x[c]                                    # [4096, 128]
        xt = np.ascontiguousarray(xs.T).astype(GEMM_NP)       # [128, 4096]
        x2 = (xs ** 2).sum(axis=1, dtype=np.float32)          # [4096]
        x2cols = np.ascontiguousarray(x2.reshape(N_TILES, P).T)  # [128, 32]
        in_maps.append({"xt": xt, "wneg2": wneg2, "x2": x2cols, "w2p": w2p})
    return in_maps


def run(x, w, trace=False):
    _install_ntff_hook()
    nc = _get_nc()
    in_maps = make_in_maps(x, w)
    last_err = None
    for _attempt in range(3):
        try:
            res = run_bass_kernel_spmd(nc, in_maps,
                                       core_ids=list(range(N_CORES)),
                                       trace=trace)
            break
        except Exception as e:  # transient device/tunnel hiccups
            last_err = e
    else:
        raise last_err
    out = np.stack([res.results[c]["out"] for c in range(N_CORES)], axis=0)
    return out.astype(np.float32), res


def kernel(x, w):
    out, _ = run(x, w, trace=False)
    return out


# revision 15
# speedup vs baseline: 1.5606x; 1.0933x over previous
"""Squared-Euclidean-distance kernel for Trainium2 (8 NeuronCores, SPMD).

Computes out[b,n,u] = sum_d (x[b,n,d] - w[d,u])^2 for
x [8, 4096, 128] f32, w [128, 1024] f32 -> out [8, 4096, 1024] f32,
via the algebraic identity |x|^2 + |w|^2 - 2 x.w.

Distribution: data-parallel over the batch dim — core c handles x[c]
([4096, 128] rows), w replicated. No cross-core communication.

Per-core device kernel (DMA-bound; ~360 GB/s/core across 16 queues):
  - GEMM in fp16 (full PE rate): PSUM = xt_tile.T @ (-2w).
  - Output written to HBM as fp16 (halves the dominant output traffic;
    elementwise error ~1e-3 of scale) and widened to f32 on the host.
  - Epilogue split so no engine exceeds the per-tile DMA budget
    (~730 ns): ScalarE activation(bias=|x|^2) converts cols [0:ACT_U)
    and VectorE does cols [ACT_U:1024) in one scalar_tensor_tensor
    ((acc + x2) + w2); then fp16 +w2 fixups on ScalarE's slice:
    GpSimd cols [0:POOL_F), VectorE the rest (GpSimd cannot read
    PSUM, so it only gets SBUF fp16 work).
  - DMAs alternate between the SP and Act hardware DGE queues.
"""

import sys
import types

try:
    import concourse.bass as bass  # noqa: F401
except ImportError:  # fresh interpreter without the repo on sys.path
    sys.path.insert(0, "/opt/trn_rl_repo")

import numpy as np

import concourse.bass as bass
import concourse.bacc as bacc
import concourse.tile as tile
import concourse.mybir as mybir
import concourse.bass_utils as bass_utils
from concourse.bass_utils import run_bass_kernel_spmd

B, N, D, U = 8, 4096, 128, 1024
N_CORES = 8
P = 128
N_TILES = N // P          # 32 n-tiles per core
U_HALF = 512              # PSUM bank = 512 f32
XT_CHUNK = 512            # xt loaded as 8 chunks of [128, 512]
ACT_U = 576               # epilogue cols on ScalarE (+ fp16 fixups)
POOL_F = 288              # fixup cols on GpSimd: [0:POOL_F)
DVE_F_LO, DVE_F_HI = POOL_F, ACT_U   # fixup cols on VectorE

GEMM_DT = mybir.dt.float16
GEMM_NP = np.float16
OUT_DT = mybir.dt.float16
OUT_NP = np.float16


def _install_ntff_hook():
    """Wire the NTFF profile hook the agent image leaves unconnected."""
    if "antenv.axon_hooks" in sys.modules:
        return
    try:
        from trn_agent_boot.trn_boot import _ntff_profile_via_ctypes
        hook = _ntff_profile_via_ctypes("/opt/axon/libaxon_pjrt.so")
    except Exception:
        hook = None
    mod = types.ModuleType("antenv.axon_hooks")
    mod.get_axon_ntff_profile_hook = lambda: hook
    mod.set_axon_ntff_profile_hook = lambda h: None
    sys.modules["antenv.axon_hooks"] = mod
    bass_utils.upload_artifacts = lambda tmpdir: f"local://{tmpdir}"


def build_bass():
    """Build + compile the per-core Bass program (SPMD, same on all cores)."""
    nc = bacc.Bacc("TRN2", target_bir_lowering=False, debug=False,
                   enable_asserts=True, num_devices=N_CORES)

    xt_ap = nc.dram_tensor("xt", [P, N], GEMM_DT, kind="ExternalInput").ap()
    wneg2_ap = nc.dram_tensor("wneg2", [P, U], GEMM_DT, kind="ExternalInput").ap()
    x2_ap = nc.dram_tensor("x2", [P, N_TILES], mybir.dt.float32,
                           kind="ExternalInput").ap()
    # |w_u|^2 broadcast to all 128 partitions, precomputed on host in fp16
    w2p_ap = nc.dram_tensor("w2p", [P, U], OUT_DT, kind="ExternalInput").ap()
    # partition-major output: out3[p, j, u] = dist[j*128 + p, u]; tiles
    # pair into one 512 KiB DMA with 4 KiB contiguous descriptors
    out_ap = nc.dram_tensor("out", [P, N_TILES, U], OUT_DT,
                            kind="ExternalOutput").ap()

    add = mybir.AluOpType.add

    with tile.TileContext(nc) as tc:
        with (
            tc.tile_pool(name="singles", bufs=1) as singles,
            tc.tile_pool(name="xchunks", bufs=N // XT_CHUNK) as xchunks,
            tc.tile_pool(name="psum", bufs=4, space="PSUM") as psum_pool,
            tc.tile_pool(name="outs", bufs=6) as out_pool,
        ):
            # Group 0 needs both wneg2 halves + xt chunk 0 + x2 + w2p2;
            # the rest of xt overlaps with compute.
            wneg2_h = []
            for h in range(U // U_HALF):
                wtile = singles.tile([P, U_HALF], GEMM_DT, tag=f"wneg2{h}",
                                     name=f"wneg2{h}")
                wneg2_h.append(wtile)
            nc.sync.dma_start(wneg2_h[0][:], wneg2_ap[:, 0:U_HALF])
            nc.scalar.dma_start(wneg2_h[1][:], wneg2_ap[:, U_HALF:U])
            xt_sbs = []
            for ci in range(N // XT_CHUNK):
                t = xchunks.tile([P, XT_CHUNK], GEMM_DT, tag=f"xt{ci}")
                xt_sbs.append(t)
            nc.sync.dma_start(xt_sbs[0][:], xt_ap[:, 0:XT_CHUNK])
            w2p_sb = singles.tile([P, U], OUT_DT, tag="w2p")
            nc.scalar.dma_start(w2p_sb[:], w2p_ap[:])
            x2_sb = singles.tile([P, N_TILES], mybir.dt.float32, tag="x2")
            nc.scalar.dma_start(x2_sb[:], x2_ap[:])
            for ci in range(1, N // XT_CHUNK):
                eng = nc.sync if ci % 2 == 0 else nc.scalar
                eng.dma_start(xt_sbs[ci][:],
                              xt_ap[:, ci * XT_CHUNK:(ci + 1) * XT_CHUNK])

            tiles_per_chunk = XT_CHUNK // P
            for g in range(N_TILES // 2):
                acc = psum_pool.tile([P, U], mybir.dt.float32, tag="acc")
                acc_b = psum_pool.tile([P, U], mybir.dt.float32, tag="acc")
                accs = (acc, acc_b)
                o2 = out_pool.tile([P, 2, U], OUT_DT, tag="o")
                for t in range(2):
                    j = 2 * g + t
                    chunk = xt_sbs[j // tiles_per_chunk]
                    col0 = (j % tiles_per_chunk) * P
                    lhsT = chunk[:, col0:col0 + P]
                    for h in range(U // U_HALF):
                        nc.tensor.matmul(
                            accs[t][:, h * U_HALF:(h + 1) * U_HALF],
                            lhsT,
                            wneg2_h[h][:],
                            start=True, stop=True,
                        )
                for t in range(2):
                    j = 2 * g + t
                    x2col = x2_sb[:, j:j + 1]
                    # ScalarE: cols [0:ACT_U) = acc + x2 (bias), f32->fp16.
                    nc.scalar.activation(
                        out=o2[:, t, 0:ACT_U], in_=accs[t][:, 0:ACT_U],
                        func=mybir.ActivationFunctionType.Identity,
                        bias=x2col, scale=1.0,
                    )
                    # VectorE: cols [ACT_U:U) = (acc + x2) + w2 in one op.
                    nc.vector.scalar_tensor_tensor(
                        out=o2[:, t, ACT_U:U], in0=accs[t][:, ACT_U:U],
                        scalar=x2col, in1=w2p_sb[:, ACT_U:U],
                        op0=add, op1=add,
                    )
                    # fp16 fixups (+w2) on the ScalarE slice (2-D APs only:
                    # 3-D/multi-block APs run far below rate on DVE/GpSimd).
                    nc.gpsimd.tensor_tensor(
                        out=o2[:, t, 0:POOL_F], in0=o2[:, t, 0:POOL_F],
                        in1=w2p_sb[:, 0:POOL_F], op=add,
                    )
                    nc.vector.tensor_tensor(
                        out=o2[:, t, DVE_F_LO:DVE_F_HI],
                        in0=o2[:, t, DVE_F_LO:DVE_F_HI],
                        in1=w2p_sb[:, DVE_F_LO:DVE_F_HI], op=add,
                    )
                eng = nc.sync if g % 2 == 0 else nc.scalar
                eng.dma_start(out_ap[:, 2 * g:2 * g + 2, :], o2[:])

    nc.compile()
    return nc


_CACHED_NC = None


def _get_nc():
    global _CACHED_NC
    if _CACHED_NC is None:
        _CACHED_NC = build_bass()
    return _CACHED_NC


def make_in_maps(x, w):
    """Host-side shard + precompute: per-core input dict list."""
    x = np.asarray(x, dtype=np.float32)
    w = np.asarray(w, dtype=np.float32)
    wneg2 = (-2.0 * w).astype(GEMM_NP)
    w2 = (w.astype(np.float64) ** 2).sum(axis=0).astype(np.float32)
    w2p = np.broadcast_to(w2.astype(OUT_NP), (P, U)).copy()     # [128, 1024]
    in_maps = []
    for c in range(N_CORES):
        xs = x[c]                                    # [4096, 128]
        xt = np.ascontiguousarray(xs.T).astype(GEMM_NP)       # [128, 4096]
        x2 = (xs ** 2).sum(axis=1, dtype=np.float32)          # [4096]
        x2cols = np.ascontiguousarray(x2.reshape(N_TILES, P).T)  # [128, 32]
        in_maps.append({"xt": xt, "wneg2": wneg2, "x2": x2cols, "w2p": w2p})
    return in_maps


def run(x, w, trace=False):
    _install_ntff_hook()
    nc = _get_nc()
    in_maps = make_in_maps(x, w)
    last_err = None
    for _attempt in range(3):
        try:
            res = run_bass_kernel_spmd(nc, in_maps,
                                       core_ids=list(range(N_CORES)),
                                       trace=trace)
            break
        except Exception as e:  # transient device/tunnel hiccups
            last_err = e
    else:
        raise last_err
    # out3[p, j, u] -> dist[j*128 + p, u], widened to f32
    outs = []
    for c in range(N_CORES):
        o3 = res.results[c]["out"]                   # [128, 32, 1024] fp16
        outs.append(o3.transpose(1, 0, 2).reshape(N, U).astype(np.float32))
    return np.stack(outs, axis=0), res


def kernel(x, w):
    out, _ = run(x, w, trace=False)
    return out
